# revision 2
# baseline (speedup 1.0000x reference)
"""Trainium2 Bass kernel for nn_Conv2dLayer_3195455668909.

Computes: conv_transpose2d(x, w, stride=2) -> 4x4 FIR (upfirdn2d) -> bias +
leaky-relu * sqrt(2) -> clamp(+-256), for x [8,512,64,64] f32,
weight [256,512,3,3], bias [256]. Output [8,256,128,128] f32.

Strategy (one batch image per NeuronCore, 8 cores):
 - Polyphase decomposition of the stride-2 transposed conv: 4 sub-convs on
   the 64x64 grid (2x2 / 2x1 / 1x2 / 1x1 taps), each as PE matmuls
   contracting over in-channels (bf16, fp32 PSUM accumulate).
 - FIR [1,3,3,1] x [1,3,3,1] = three 2-tap box filters per axis: 6 shifted
   tensor-adds on DVE over column-phase-separated row stacks.
 - Epilogue: leaky-relu + interleave (ACT), clamp (DVE), bf16 out in the
   final [oc, o, half, t, u] layout so the host gather is a pure reshape.
All weight scaling (weight_gain, FIR normalization, act gain) is folded
into the weights/bias on the host.

Runtime: a process-level cached jit (shard_map over 8 cores) built once;
weights/bias staged on device once; x staged per call via a preallocated
padded bf16 buffer. The BIR ExternalOutput needs no operand (the lowering
allocates fresh HBM for outputs), so nothing but x moves per call.
"""
import math
from contextlib import ExitStack

import numpy as np
import ml_dtypes

import json

import jax

import concourse.bass as bass
import concourse.tile as tile
from concourse import bass2jax, mybir

N_CORES = 8
CI, CO, H, W = 512, 256, 64, 64
NIC, NOC = CI // 128, CO // 128   # channel chunks
XF = 66 * 66 + 8                  # padded-x flat length per channel (+slack)
NSLOT = 68                        # slots per fine row in a stack
NROW = 132                        # stack rows (fine row f -> stack row f+1)
LH = 131 * NSLOT                  # flat length for H-stage ops
CLAMP = 256.0
SLOPE = 0.2
ROWTAPS = {0: [(0, 0), (1, 2)], 1: [(0, 1)]}   # row-phase -> [(a', w_row)]
COLTAPS = {0: [(0, 0), (1, 2)], 1: [(0, 1)]}   # col-phase -> [(b', w_col)]
BF16 = mybir.dt.bfloat16
F32 = mybir.dt.float32


def _split_multi_waits(bir_bytes):
    """The walrus build here rejects instructions with more than one sync
    wait. Move extra waits onto same-engine NoOps inserted just before."""
    d = json.loads(bir_bytes)
    for fn in d["functions"]:
        for blk in fn["blocks"]:
            insts = blk.get("instructions")
            if not insts:
                continue
            out = []
            for ins in insts:
                si = ins.get("sync_info") or {}
                waits = si.get("on_wait") or []
                if len(waits) > 1:
                    for i, w in enumerate(waits[1:]):
                        out.append({
                            "debug": ins.get("debug", 0),
                            "engine": ins["engine"],
                            "ins": [],
                            "name": f"{ins['name']}-xw{i}",
                            "opcode": "NoOp",
                            "outs": [],
                            "sync_info": {"on_update": [], "on_wait": [w]},
                        })
                    si["on_wait"] = waits[:1]
                out.append(ins)
            blk["instructions"] = out
    return json.dumps(d).encode()


_orig_compile_bir_kernel = bass2jax.compile_bir_kernel


def _patched_compile_bir_kernel(ant_bir_str, *args, **kwargs):
    return _orig_compile_bir_kernel(_split_multi_waits(ant_bir_str), *args, **kwargs)


if bass2jax.compile_bir_kernel is not _patched_compile_bir_kernel:
    bass2jax.compile_bir_kernel = _patched_compile_bir_kernel


def _build_program():
    nc = bass.Bass()
    xp_d = nc.declare_dram_parameter("xp", [NIC, 128, XF], BF16, isOutput=False)
    wt_d = nc.declare_dram_parameter("wt", [NIC, 128, 3 * 3 * NOC * 128], BF16,
                                     isOutput=False)
    bs_d = nc.declare_dram_parameter("bs", [128, NOC], F32, isOutput=False)
    zo_d = nc.declare_dram_parameter("zo", [NOC, 128, 2 * 64 * 128], BF16,
                                     isOutput=True)

    ctx = ExitStack()
    with ctx:
        tc = ctx.enter_context(tile.TileContext(nc))
        const = ctx.enter_context(tc.tile_pool(name="const", bufs=1))
        psum = ctx.enter_context(tc.tile_pool(name="psum", bufs=6, space="PSUM"))
        stks = ctx.enter_context(tc.tile_pool(name="stks", bufs=2))
        zp = ctx.enter_context(tc.tile_pool(name="zp", bufs=2))

        x_sb = const.tile([128, NIC, XF], BF16)
        w_sb = const.tile([128, NIC, 3, 3, NOC, 128], BF16)
        b_sb = const.tile([128, NOC], F32)
        for ic in range(NIC):
            nc.sync.dma_start(x_sb[:, ic], xp_d[ic])
            nc.sync.dma_start(
                w_sb[:, ic].rearrange("p a b o m -> p (a b o m)"), wt_d[ic]
            )
        nc.sync.dma_start(b_sb[:], bs_d[:])

        for oc in range(NOC):
            yE = stks.tile([128, NROW, NSLOT], BF16, tag="yE")
            yO = stks.tile([128, NROW, NSLOT], BF16, tag="yO")
            A = stks.tile([128, NROW, NSLOT], BF16, tag="A")
            nc.vector.memset(yE[:], 0.0)
            nc.vector.memset(yO[:], 0.0)
            stk = {0: yE, 1: yO}

            # --- conv: polyphase matmuls, accumulate taps x in-chunks ---
            for rp in (0, 1):
                nrows = 65 if rp == 0 else 64
                for cp in (0, 1):
                    taps = [(a_, wa, b_, wb)
                            for (a_, wa) in ROWTAPS[rp]
                            for (b_, wb) in COLTAPS[cp]]
                    for P0 in range(0, nrows, 7):
                        R = min(7, nrows - P0)
                        acc = psum.tile([128, R * 66], F32, tag="acc")
                        n = NIC * len(taps)
                        k = 0
                        for ic in range(NIC):
                            for (a_, wa, b_, wb) in taps:
                                start = (P0 + 1 - a_) * 66 + (1 - b_)
                                nc.tensor.matmul(
                                    acc[:],
                                    w_sb[:, ic, wa, wb, oc, :],
                                    x_sb[:, ic, start:start + R * 66],
                                    start=(k == 0), stop=(k == n - 1),
                                )
                                k += 1
                        r0 = 1 + rp + 2 * P0
                        nc.scalar.copy(
                            stk[cp][:, r0:r0 + 2 * R:2, 2:68],
                            acc[:].rearrange("p (r c) -> p r c", c=66),
                        )
            # zero the garbage cols of yO (phase cols Q=64,65 are invalid)
            nc.vector.memset(yO[:, :, 66:68], 0.0)

            yEf = yE[:].rearrange("p a b -> p (a b)")
            yOf = yO[:].rearrange("p a b -> p (a b)")
            Af = A[:].rearrange("p a b -> p (a b)")

            # --- H FIR: 3 box passes, col-phase separated ---
            def eop(dst, p, q):   # dst[s] = p[s] + q[s]
                nc.vector.tensor_add(dst[:, :LH], p[:, :LH], q[:, :LH])

            def oop(q, p):        # q[s] = q[s] + p[s+1]
                nc.vector.tensor_add(q[:, :LH], q[:, :LH], p[:, 1:LH + 1])

            eop(Af, yEf, yOf); oop(yOf, yEf)
            eop(yEf, Af, yOf); oop(yOf, Af)
            eop(Af, yEf, yOf); oop(yOf, yEf)
            # hE in A, hO in yO, scratch = yE

            # --- V FIR: 3 box passes, ping-pong (row shift = NSLOT elems) ---
            def vpass(dst, src, rows_out):
                m = rows_out * NSLOT
                nc.vector.tensor_add(
                    dst[:, :m], src[:, :m], src[:, NSLOT:m + NSLOT]
                )

            vpass(yEf, Af, 130); vpass(Af, yEf, 129); vpass(yEf, Af, 128)
            FE = yE   # z row t at stack row t; z[t,2T+1] = FE[t, T+2]
            vpass(Af, yOf, 130); vpass(yOf, Af, 129); vpass(Af, yOf, 128)
            FO = A    # z[t,2T] = FO[t, T+1]

            # --- epilogue: lrelu + interleave (ACT), clamp (DVE), DMA out ---
            for half in range(2):
                t0 = 64 * half
                Z = zp.tile([128, 64, 128], BF16, tag="Z")
                nc.scalar.activation(
                    Z[:, :, 0:128:2], FO[:, t0:t0 + 64, 1:65],
                    mybir.ActivationFunctionType.Identity,
                    bias=b_sb[:, oc:oc + 1], scale=1.0,
                )
                nc.scalar.activation(
                    Z[:, :, 1:128:2], FE[:, t0:t0 + 64, 2:66],
                    mybir.ActivationFunctionType.Identity,
                    bias=b_sb[:, oc:oc + 1], scale=1.0,
                )
                Zf = Z[:].rearrange("p a b -> p (a b)")
                # leaky relu: z = max(0.2*z, z), then clamp to +-256
                nc.vector.scalar_tensor_tensor(
                    Zf, Zf, SLOPE, Zf,
                    mybir.AluOpType.mult, mybir.AluOpType.max,
                )
                nc.vector.tensor_scalar(
                    Zf, Zf, CLAMP, -CLAMP,
                    mybir.AluOpType.min, mybir.AluOpType.max,
                )
                nc.sync.dma_start(
                    zo_d[oc, :, half * 8192:(half + 1) * 8192], Zf
                )
    return nc


_RT: dict = {}


def _runtime():
    """Build the Bass program + cached jit once per process."""
    if "jf" in _RT:
        return _RT
    from jax.sharding import Mesh, PartitionSpec, NamedSharding
    from jax.experimental.shard_map import shard_map

    bass2jax.install_neuronx_cc_hook()
    nc = _build_program()

    partition_name = (nc.partition_id_tensor.name
                      if nc.partition_id_tensor else None)
    in_names, out_names, out_avals = [], [], []
    for alloc in nc.m.functions[0].allocations:
        if not isinstance(alloc, mybir.MemoryLocationSet):
            continue
        name = alloc.memorylocations[0].name
        if alloc.kind == "ExternalInput":
            if name != partition_name:
                in_names.append(name)
        elif alloc.kind == "ExternalOutput":
            out_names.append(name)
            out_avals.append(jax.core.ShapedArray(
                tuple(alloc.tensor_shape), mybir.dt.np(alloc.dtype)))
    in_names_all = list(in_names)
    if partition_name:
        in_names_all.append(partition_name)

    def _body(*args):
        operands = list(args)
        if partition_name:
            operands.append(bass2jax.partition_id_tensor())
        # ExternalOutputs need no operands: with no input/output aliases the
        # lowering allocates fresh shared_hbm buffers, and this kernel
        # writes every output element.
        return tuple(bass2jax._bass_exec_p.bind(
            *operands,
            out_avals=tuple(out_avals),
            in_names=tuple(in_names_all),
            out_names=tuple(out_names),
            lowering_input_output_aliases=(),
            sim_require_finite=True,
            sim_require_nnan=True,
            nc=nc,
        ))

    devices = jax.devices()[:N_CORES]
    mesh = Mesh(np.asarray(devices), ("core",))
    shard = NamedSharding(mesh, PartitionSpec("core"))
    jf = jax.jit(
        shard_map(_body, mesh=mesh,
                  in_specs=(PartitionSpec("core"),) * len(in_names),
                  out_specs=(PartitionSpec("core"),) * len(out_names),
                  check_rep=False),
        keep_unused=True,
    )

    xbuf = np.zeros((N_CORES * NIC, 128, XF), ml_dtypes.bfloat16)
    _RT.update(jf=jf, in_names=in_names, shard=shard, xbuf=xbuf, wkey=None)
    return _RT


def _stage_weights(weight, bias):
    """Device-put the (layer-constant) transformed weights, cached."""
    rt = _runtime()
    key = (id(weight), id(bias))
    if rt["wkey"] == key:
        return
    scale = math.sqrt(2.0) / (math.sqrt(CI * 9) * 16.0)
    w = (np.asarray(weight, np.float32) * scale)
    # [4 ic, 128 i, 3 a, 3 b, 2 oc, 128 o]
    wt = np.ascontiguousarray(
        w.reshape(NOC, 128, NIC, 128, 3, 3).transpose(2, 3, 4, 5, 0, 1)
    ).reshape(NIC, 128, 3 * 3 * NOC * 128).astype(ml_dtypes.bfloat16)
    b = (np.asarray(bias, np.float32) * math.sqrt(2.0)).reshape(NOC, 128)
    bs = np.ascontiguousarray(b.T).astype(np.float32)  # [128, NOC]
    rt["dev_wt"] = jax.device_put(
        np.tile(wt, (N_CORES, 1, 1)), rt["shard"])
    rt["dev_bs"] = jax.device_put(
        np.tile(bs, (N_CORES, 1)), rt["shard"])
    jax.block_until_ready((rt["dev_wt"], rt["dev_bs"]))
    rt["wkey"] = key


def _stage_x(x):
    """Pad + cast x into the persistent staging buffer, ship to devices."""
    rt = _runtime()
    xbuf = rt["xbuf"]
    v = xbuf.reshape(N_CORES, CI, XF)[:, :, :66 * 66]
    v.reshape(N_CORES, CI, 66, 66)[:, :, 1:65, 1:65] = x
    return jax.device_put(xbuf, rt["shard"])


def _dispatch(dx):
    rt = _RT
    args = {"xp": dx, "wt": rt["dev_wt"], "bs": rt["dev_bs"]}
    return rt["jf"](*[args[n] for n in rt["in_names"]])


def _gather(zo):
    host = np.asarray(zo)  # bf16 [8*NOC, 128, 2*64*128]
    return host.reshape(N_CORES, CO, 128, 128).astype(np.float32)


def kernel(x, weight, bias):
    _stage_weights(weight, bias)
    dx = _stage_x(np.asarray(x, np.float32))
    (zo,) = _dispatch(dx)
    return _gather(zo)


def bench_exec(x, weight, bias, iters=50):
    """Amortized per-call wall time (ns) of the compiled 8-core kernel with
    all operands device-resident — the closest available proxy for HW exec
    time (NTFF profiling is unavailable under this axon build). Upper bound:
    includes per-call axon RPC dispatch."""
    import time
    _stage_weights(weight, bias)
    dx = _stage_x(np.asarray(x, np.float32))
    out = None
    for _ in range(3):  # warmup
        out = _dispatch(dx)
    jax.block_until_ready(out)
    t0 = time.perf_counter()
    for _ in range(iters):
        out = _dispatch(dx)
    jax.block_until_ready(out)
    dt = time.perf_counter() - t0
    return int(dt / iters * 1e9)
